# revision 77
# baseline (speedup 1.0000x reference)
"""Causal self-attention (B=2, T=2048, C=1024, H=16, RoPE) on 8 TRN2 cores.

Sharding: data-parallel over B (2 groups of 4 cores) x tensor-parallel over
heads (4 heads per core). Each core computes q/k/v projections for its heads,
RoPE, causal attention, and its partial output projection; the host sums the
4 partial projections per batch and adds bp.

Layout choices (per core):
  - xT [C, T] resident in SBUF (contraction dim C on partitions), loaded as 8
    per-chunk DMAs so the first projection matmul starts as soon as chunk 0
    lands; weights are host-pre-shuffled into [128, ...] layouts so every
    weight DMA is a cheap 2D transfer.
  - q, k produced TRANSPOSED: qT/kT [256=4heads*64, T] via lhsT=W, rhs=xT.
    Head-dim pairs are pre-permuted (evens|odds) in the weights so RoPE
    needs no strided access; the pair-swap is a constant permutation
    matmul (J), combine on VectorE with bf16 cos/sin.
  - v produced NON-transposed: [T, 256] via lhsT=xT, rhs=WvT.
  - scores computed transposed: ST[tk, tq] = k_rot @ q_rot^T per head; the
    two heads of a pair run CONCURRENTLY on the PE array (row-tiled, K=64
    each at row groups 0/64). The causal triangle on diagonal blocks is
    applied IN PSUM by accumulating a constant NEG upper-triangle matmul
    (ntri @ eye) on top of the scores, so softmax-exp (ScalarE, scale=1/8
    folded in) produces exact zeros there and nothing sits between exp and
    PV on the critical path. Fully-masked left columns of diagonal blocks
    are simply never computed or read; fully-masked blocks are skipped.
  - P@V also paired: M=64 per head (col groups 0/64) into one PSUM tile.
  - softmax denominators accumulate on VectorE; a pair of 1-row
    ones-matmuls per group reduces over partitions (col-group paired);
    EA-matmul broadcasts back; reciprocal_approx_fast + scale on VectorE.
  - Attention groups run head-pair 0 first (all query blocks), then head
    pair 1, so attention starts right after jc0's projections while jc1's
    projections/RoPE, the remaining v tiles, and the output projections
    drip-feed into attention's PE gaps (scores(kc+1) is also emitted
    before PV(kc) so the PE never stalls behind the current exp).
  - output projection per qb; PSUM evacuated by VectorE/ScalarE into a
    staging tile, one batched DMA per qb writes z; the final qb is
    normalized, projected and written out per 128-row block so the tail
    pipelines.
"""

import math

import numpy as np
import ml_dtypes

import concourse.bass as bass
import concourse.bacc as bacc
import concourse.mybir as mybir
from concourse.tile import TileContext
from concourse.bass_utils import run_bass_kernel_spmd

BF16 = mybir.dt.bfloat16
F32 = mybir.dt.float32
NPBF16 = ml_dtypes.bfloat16

N_CORES = 8
P = 128
NEG = -1e9

_UNIFIED_ACT_SET = "natural_log_exp_and_others"


def _patch_act_tables():
    """Make Exp and Copy resolve to one activation table (no reload churn)."""
    import concourse.hw_specs as _hw
    import concourse.bacc as _bacc
    if getattr(_bacc, "_act_tables_patched", False):
        return
    _orig = _hw.get_activation_tables

    def _gat(arch):
        tabs = _orig(arch)
        if _UNIFIED_ACT_SET in tabs:
            keep = tabs[_UNIFIED_ACT_SET]
            drop = {
                mybir.ActivationFunctionType.Exp,
                mybir.ActivationFunctionType.Copy,
            } & keep
            for name, fns in tabs.items():
                if name != _UNIFIED_ACT_SET:
                    for f in drop:
                        fns.discard(f)
        return tabs

    _bacc.get_activation_tables = _gat
    _bacc._act_tables_patched = True


def build_attention_kernel(nc, T=2048, C=1024, n_heads=4, hd=64,
                           zero_qk_bias=False):
    """Emit the per-core kernel. Returns nothing; tensors are declared on nc.

    zero_qk_bias: host-verified hint that bq/bk are all zero, letting the
    q/k PSUM evacuation run as a plain copy on the (idle) ScalarE instead
    of a bias-add on the saturated VectorE.
    """
    _patch_act_tables()
    HD = n_heads * hd            # 256: local head dims
    KC = C // P                  # 8: contraction chunks for projections
    NJC = HD // P                # 2: partition tiles of qT/kT (head pairs)
    TQB = 512                    # tq block for scores/PV
    NQB = T // TQB               # 4
    NTT = T // P                 # 16: t tiles for v
    scale = 1.0 / math.sqrt(hd)

    # ---- DRAM I/O (weights host-pre-shuffled to [128, .] layouts) ----
    xT = nc.declare_dram_parameter("xT", [C, T], BF16, isOutput=False)
    wqs = nc.declare_dram_parameter("wqs", [P, KC * HD], BF16, isOutput=False)
    wks = nc.declare_dram_parameter("wks", [P, KC * HD], BF16, isOutput=False)
    wvs = nc.declare_dram_parameter("wvs", [P, KC * HD], BF16, isOutput=False)
    wps = nc.declare_dram_parameter("wps", [P, NJC * C], BF16, isOutput=False)
    # f32 consts: bq(NJC) | bk(NJC) | bv(HD)
    NFC = 2 * NJC + HD
    fcst = nc.declare_dram_parameter("fcst", [P, NFC], F32, isOutput=False)
    # bf16 consts: j | ea | ntri | eye | ones(2); cos/sin ship as their
    # periodic cores (32/64 rows) and are replicated on-chip
    NBC = 4 * P + 2
    bcst = nc.declare_dram_parameter("bcst", [P, NBC], BF16, isOutput=False)
    cosd = nc.declare_dram_parameter("cosd", [32, T], BF16, isOutput=False)
    sind = nc.declare_dram_parameter("sind", [64, T], BF16, isOutput=False)
    z = nc.declare_dram_parameter("z", [T, C], F32, isOutput=True)

    with TileContext(nc) as tc:
        import contextlib

        with contextlib.ExitStack() as ctx:
            # ---- persistent SBUF pools ----
            pc = ctx.enter_context(tc.tile_pool(name="const", bufs=1))
            px = ctx.enter_context(tc.tile_pool(name="x", bufs=1))
            pw = ctx.enter_context(tc.tile_pool(name="w", bufs=1))
            pqk = ctx.enter_context(tc.tile_pool(name="qk", bufs=1))
            pv = ctx.enter_context(tc.tile_pool(name="v", bufs=1))
            py = ctx.enter_context(tc.tile_pool(name="y", bufs=1))
            # transient pools
            praw = ctx.enter_context(tc.tile_pool(name="raw", bufs=2))
            pswp = ctx.enter_context(tc.tile_pool(name="swp", bufs=2))
            prt = ctx.enter_context(tc.tile_pool(name="ropetmp", bufs=6))
            pexp = ctx.enter_context(tc.tile_pool(name="exp", bufs=8))
            prcp = ctx.enter_context(tc.tile_pool(name="rcp", bufs=2))
            pzs = ctx.enter_context(tc.tile_pool(name="zstage", bufs=2))
            # transient pool for the denominator accumulator
            pacc = ctx.enter_context(tc.tile_pool(name="acc", bufs=2))
            # PSUM pools (4 + 2 + 2 = 8 banks)
            psc = ctx.enter_context(
                tc.tile_pool(name="sc", bufs=2, space="PSUM"))
            pyt = ctx.enter_context(
                tc.tile_pool(name="yt", bufs=2, space="PSUM"))
            pz = ctx.enter_context(
                tc.tile_pool(name="zp", bufs=2, space="PSUM"))

            # ---- input DMAs: x split over two queues, consumed k-outer;
            # later-needed weights queue BEHIND x so they don't steal HBM
            # bandwidth from the critical x stream ----
            # wq's first chunk ships alone so the very first projection
            # matmul fires as soon as x0 lands
            t_wq = pw.tile([P, KC * HD], BF16, tag="wq")
            nc.scalar.dma_start(t_wq[:, 0:HD], wqs[:, 0:HD])
            nc.scalar.dma_start(t_wq[:, HD:], wqs[:, HD:])
            t_wk = pw.tile([P, KC * HD], BF16, tag="wk")
            nc.scalar.dma_start(t_wk[:], wks[:])
            t_x = [px.tile([P, T], BF16, tag=f"x{k}", name=f"x{k}")
                   for k in range(KC)]
            for k in range(0, KC, 2):
                nc.sync.dma_start(t_x[k][:], xT[k * P:(k + 1) * P, :])
                nc.scalar.dma_start(
                    t_x[k + 1][:], xT[(k + 1) * P:(k + 2) * P, :])
            t_fc = pc.tile([P, NFC], F32, tag="fc")
            nc.scalar.dma_start(t_fc[:], fcst[:])
            t_bc = pc.tile([P, NBC], BF16, tag="bc")
            nc.gpsimd.dma_start(t_bc[:], bcst[:])
            # cos/sin: one small HBM read + SBUF-to-SBUF partition
            # replication (saves 0.75MB of the critical input stream)
            t_cos = pc.tile([P, T], BF16, tag="cos")
            nc.gpsimd.dma_start(t_cos[0:32, :], cosd[:])
            t_sin = pc.tile([P, T], BF16, tag="sin")
            nc.gpsimd.dma_start(t_sin[0:64, :], sind[:])
            nc.gpsimd.dma_start(t_cos[32:64, :], t_cos[0:32, :])
            nc.sync.dma_start(t_cos[64:96, :], t_cos[0:32, :])
            nc.gpsimd.dma_start(t_cos[96:128, :], t_cos[0:32, :])
            nc.sync.dma_start(t_sin[64:128, :], t_sin[0:64, :])
            t_wv = pw.tile([P, KC * HD], BF16, tag="wv")
            nc.sync.dma_start(t_wv[:], wvs[:])
            t_wp = pw.tile([P, NJC * C], BF16, tag="wp")
            nc.gpsimd.dma_start(t_wp[:], wps[:])

            # const views
            t_j = t_bc[:, 0:P]
            t_ea = t_bc[:, P:2 * P]
            t_ntri = t_bc[:, 2 * P:3 * P]
            t_eye = t_bc[:, 3 * P:4 * P]
            t_ones = t_bc[:, 4 * P:4 * P + 1]
            cos = t_cos[:]
            sin = t_sin[:]
            bq = t_fc[:, 0:NJC]
            bk = t_fc[:, NJC:2 * NJC]
            bv = t_fc[:, 2 * NJC:2 * NJC + HD]

            # persistent staging tile for softmax denominators (rows 0/64
            # carry data; the rest must be finite zeros for the EA matmul)
            t_scp = pc.tile([P, TQB], BF16, tag="scp")
            nc.vector.memset(t_scp[:], 0.0)

            # ---- q/k projections (transposed) + RoPE ----
            t_qrot = [pqk.tile([P, T], BF16, tag=f"qr{jc}", name=f"qrot{jc}")
                      for jc in range(NJC)]
            t_krot = [pqk.tile([P, T], BF16, tag=f"kr{jc}", name=f"krot{jc}")
                      for jc in range(NJC)]

            def qk_evac(dst, src, bias_col):
                if zero_qk_bias:
                    nc.scalar.copy(dst, src)
                else:
                    nc.vector.tensor_scalar_add(dst, src, bias_col)

            def emit_swap(raw, swp, sl, eng0, eng1):
                """swp[sl] = (J@raw)[sl]: the RoPE pair-swap is a pure
                32-partition block permutation, done by 4 SBUF-to-SBUF DMA
                copies on otherwise-idle queues instead of PE matmuls."""
                for b, eng in zip(range(4), (eng0, eng1, eng0, eng1)):
                    sb = b ^ 1  # 0<->1, 2<->3 (32-row block pair swap)
                    eng.dma_start(
                        swp[b * 32:(b + 1) * 32, sl],
                        raw[sb * 32:(sb + 1) * 32, sl])

            def rope_block(raw, swp, dst, sl):
                """rot[sl] = cos[sl]*raw[sl] + sin[sl]*swp[sl]."""
                W = sl.stop - sl.start
                tmp1 = prt.tile([P, W], BF16, tag="rope1")
                nc.vector.tensor_mul(tmp1[:], raw[:, sl], cos[:, sl])
                tmp2 = prt.tile([P, W], BF16, tag="rope2")
                with nc.allow_low_precision(reason="bf16 rope"):
                    nc.vector.tensor_mul(tmp2[:], swp[:, sl], sin[:, sl])
                    nc.vector.tensor_add(dst[:, sl], tmp1[:], tmp2[:])

            # ---- jc0 q/k projections + RoPE, dense (x streams in) ----
            first_group = True
            for (wt, bias, dst) in (
                (t_wq, bq, t_qrot[0]),
                (t_wk, bk, t_krot[0]),
            ):
                raw = praw.tile([P, T], BF16, tag="qkraw")
                if first_group:
                    # k-outer: each x chunk is consumed as soon as its
                    # DMA lands; 4 tb accumulators in 2 two-bank tiles.
                    # The end-of-group evac bubble hides behind the wk
                    # DMA this group would wait for anyway.
                    first_group = False
                    qp = [psc.tile([P, 2 * TQB], F32, tag="sc",
                                   name=f"qp{i}") for i in range(2)]
                    for k in range(KC):
                        for tb in range(T // TQB):
                            nc.tensor.matmul(
                                qp[tb // 2][:, (tb % 2) * TQB:
                                            (tb % 2 + 1) * TQB],
                                lhsT=wt[:, k * HD:k * HD + P],
                                rhs=t_x[k][:, tb * TQB:(tb + 1) * TQB],
                                start=(k == 0),
                                stop=(k == KC - 1),
                                skip_group_check=True,
                            )
                    for tb in range(T // TQB):
                        qk_evac(
                            raw[:, tb * TQB:(tb + 1) * TQB],
                            qp[tb // 2][:, (tb % 2) * TQB:
                                        (tb % 2 + 1) * TQB],
                            bias[:, 0:1],
                        )
                    swp = pswp.tile([P, T], BF16, tag="swp")
                    emit_swap(raw, swp, slice(0, T), nc.gpsimd, nc.sync)
                    for half in range(2):
                        rope_block(
                            raw, swp, dst,
                            slice(half * 1024, (half + 1) * 1024))
                else:
                    # tb-outer with swap+RoPE interleaved per 512 block,
                    # so the first attention scores can fire as soon as
                    # k's first block is rotated
                    swp = pswp.tile([P, T], BF16, tag="swp")
                    for tb in range(T // TQB):
                        qps = psc.tile([P, 2 * TQB], F32, tag="sc")
                        for k in range(KC):
                            nc.tensor.matmul(
                                qps[:, 0:TQB],
                                lhsT=wt[:, k * HD:k * HD + P],
                                rhs=t_x[k][:, tb * TQB:(tb + 1) * TQB],
                                start=(k == 0),
                                stop=(k == KC - 1),
                            )
                        sl = slice(tb * TQB, (tb + 1) * TQB)
                        qk_evac(raw[:, sl], qps[:, 0:TQB], bias[:, 0:1])
                        emit_swap(raw, swp, sl, nc.gpsimd, nc.sync)
                        rope_block(raw, swp, dst, sl)

            # ---- jc1 q/k projections + RoPE, as drip-fed thunks ----
            jc1_raw = {}

            def jc1_proj_thunk(mat, tb):
                def emit():
                    wt, bias = ((t_wq, bq) if mat == 'q' else (t_wk, bk))
                    if tb == 0:
                        jc1_raw[mat] = praw.tile(
                            [P, T], BF16, tag="qkraw", name=f"raw1{mat}")
                    raw = jc1_raw[mat]
                    qps = pz.tile([P, TQB], F32, tag="zp")
                    for k in range(KC):
                        nc.tensor.matmul(
                            qps[:],
                            lhsT=wt[:, k * HD + P:(k + 1) * HD],
                            rhs=t_x[k][:, tb * TQB:(tb + 1) * TQB],
                            start=(k == 0),
                            stop=(k == KC - 1),
                        )
                    qk_evac(
                        raw[:, tb * TQB:(tb + 1) * TQB], qps[:], bias[:, 1:2])
                return emit

            jc1_swp = {}

            def jc1_rope_thunk(mat, tb):
                def emit():
                    dst = t_qrot[1] if mat == 'q' else t_krot[1]
                    if tb == 0:
                        jc1_swp[mat] = pswp.tile(
                            [P, T], BF16, tag="swp", name=f"swp1{mat}")
                        emit_swap(jc1_raw[mat], jc1_swp[mat],
                                  slice(0, T), nc.gpsimd, nc.sync)
                    rope_block(
                        jc1_raw[mat], jc1_swp[mat], dst,
                        slice(tb * TQB, (tb + 1) * TQB))
                return emit

            # ---- y_norm accumulators ----
            t_yn = [py.tile([P, T], BF16, tag=f"yn{jc}", name=f"yn{jc}")
                    for jc in range(NJC)]

            # ---- deferred-work emitters (drip-fed between attention kcs) ---
            t_v = [None] * NTT

            def vproj_thunk(tt):
                def emit():
                    vps = pz.tile([P, TQB], F32, tag="zp")
                    for k in range(KC):
                        nc.tensor.matmul(
                            vps[:, 0:HD],
                            lhsT=t_x[k][:, tt * P:(tt + 1) * P],
                            rhs=t_wv[:, k * HD:(k + 1) * HD],
                            start=(k == 0),
                            stop=(k == KC - 1),
                        )
                    v_t = pv.tile([P, HD], BF16, tag=f"v{tt}")
                    nc.vector.tensor_add(v_t[:], vps[:, 0:HD], bv)
                    t_v[tt] = v_t
                return emit

            def zproj_thunks(qb):
                """Output projection for query block qb, as 9 thunks."""
                zs = pzs.tile([P, (TQB // P) * C], F32, tag="zs")
                thunks = []

                def group(m, co):
                    def emit():
                        tt = qb * (TQB // P) + m
                        zps = pz.tile([P, TQB], F32, tag="zp")
                        for jc in range(NJC):
                            nc.tensor.matmul(
                                zps[:],
                                lhsT=t_yn[jc][:, tt * P:(tt + 1) * P],
                                rhs=t_wp[:, jc * C + co * TQB:
                                         jc * C + (co + 1) * TQB],
                                start=(jc == 0),
                                stop=(jc == NJC - 1),
                            )
                        # evacuation split between VectorE and ScalarE
                        eng = nc.vector.tensor_copy if co == 0 \
                            else nc.scalar.copy
                        eng(
                            zs[:, (m * (C // TQB) + co) * TQB:
                               (m * (C // TQB) + co + 1) * TQB],
                            zps[:])
                    return emit

                for m in range(TQB // P):
                    for co in range(C // TQB):
                        thunks.append(group(m, co))

                def dma():
                    nc.gpsimd.dma_start(
                        z[qb * TQB:(qb + 1) * TQB, :].rearrange(
                            "(m p) (co c) -> p m co c",
                            m=TQB // P, co=C // TQB),
                        zs[:].rearrange(
                            "p (m co c) -> p m co c",
                            m=TQB // P, co=C // TQB),
                    )
                thunks.append(dma)
                return thunks

            # first v tiles dense (needed immediately by (qb=0, hp=0));
            # the rest plus all of jc1's projections drip into h0 attention
            for tt in range(TQB // P):
                vproj_thunk(tt)()

            pending = [('v', vproj_thunk(tt)) for tt in range(TQB // P, NTT)]
            pending += [('j', jc1_proj_thunk('q', tb))
                        for tb in range(T // TQB)]
            pending += [('j', jc1_rope_thunk('q', tb))
                        for tb in range(T // TQB)]
            pending += [('j', jc1_proj_thunk('k', tb))
                        for tb in range(T // TQB)]
            pending += [('j', jc1_rope_thunk('k', tb))
                        for tb in range(T // TQB)]

            def pop_pending(tags):
                for i, (tag, th) in enumerate(pending):
                    if tag in tags:
                        pending.pop(i)
                        th()
                        return True
                return False

            # stride-0 head-broadcast view of eye for the NTRI accumulate
            eye2 = bass.AP(
                t_eye.tensor, t_eye.offset,
                [t_eye.ap[0], [0, 2], t_eye.ap[1]])

            def emit_scores(qb, hp, kc):
                """Scores pair (+ causal NEG triangle) for one tk block."""
                sc = psc.tile([P, 2 * TQB], F32, tag="sc")
                sc3 = sc[:].rearrange("p (h w) -> p h w", h=2)
                s0 = max(0, kc * P - qb * TQB)
                for hl in range(2):
                    nc.tensor.matmul(
                        sc3[:, hl, s0:TQB],
                        lhsT=t_krot[hp][
                            hl * hd:(hl + 1) * hd,
                            kc * P:(kc + 1) * P],
                        rhs=t_qrot[hp][
                            hl * hd:(hl + 1) * hd,
                            qb * TQB + s0:(qb + 1) * TQB],
                    )
                if kc * P >= qb * TQB:
                    # causal triangle: accumulate NEG upper-triangle onto
                    # the diagonal 128x128 block (both heads, one matmul)
                    nc.tensor.matmul(
                        sc3[:, :, s0:s0 + P],
                        lhsT=t_ntri,
                        rhs=eye2,
                        start=False,
                        stop=True,
                        skip_group_check=True,
                    )
                return sc

            # ---- attention groups (all hp=0 first, then hp=1),
            # software-pipelined one group ahead ----
            groups = [(qb, hp) for hp in range(NJC) for qb in range(NQB)]
            carried_sc = None
            # deferred epilogue tail (EA broadcast + reciprocal + y scale)
            # of the PREVIOUS group: fired a few kc into the next group so
            # the PE never stalls on the staging copies at the boundary
            fin_prev = [None]
            for gi, (qb, hp) in enumerate(groups):
                if hp == 0:
                    # force out any v tiles this qb's PV loop will read
                    while t_v[(qb + 1) * (TQB // P) - 1] is None:
                        pop_pending(('v',))
                # early short groups drip only v tiles; jc1 projections are
                # held back for the long thunk-starved qb>=2 groups
                tags = ('v',) if (hp == 0 and qb < 2) else ('v', 'j', 'z')
                n_kc = (qb + 1) * (TQB // P)
                scs = [None] * n_kc
                scs[0] = carried_sc if carried_sc is not None \
                    else emit_scores(qb, hp, 0)
                carried_sc = None
                yt = pyt.tile([P, TQB], F32, tag="yt")
                acc = pacc.tile([P, 2 * TQB], BF16, tag="acc")
                acc3 = acc[:].rearrange("p (h w) -> p h w", h=2)
                for kc in range(n_kc):
                    sc = scs[kc]
                    # exp with 1/sqrt(hd) folded in; left cols of diagonal
                    # blocks are zeroed on GpSimd (hidden behind exp).
                    # kc==0 writes straight into the denominator
                    # accumulator, saving the init copy.
                    s0 = max(0, kc * P - qb * TQB)
                    ex = pexp.tile([P, 2 * TQB], BF16, tag="exp")
                    sc3 = sc[:].rearrange("p (h w) -> p h w", h=2)
                    ex3 = ex[:].rearrange("p (h w) -> p h w", h=2)
                    nc.scalar.activation(
                        ex3[:, :, s0:TQB],
                        sc3[:, :, s0:TQB],
                        mybir.ActivationFunctionType.Exp,
                        scale=scale,
                    )
                    if kc + 1 < n_kc:
                        scs[kc + 1] = emit_scores(qb, hp, kc + 1)
                    elif gi + 1 < len(groups):
                        # prefetch the next group's first scores so exp
                        # there overlaps this group's epilogue
                        carried_sc = emit_scores(*groups[gi + 1], 0)
                    if kc == 2 and fin_prev[0] is not None:
                        fin_prev[0]()
                        fin_prev[0] = None
                    # denominator partial sums on VectorE (both heads)
                    if kc == 0:
                        nc.vector.tensor_copy(acc[:], ex[:])
                    else:
                        nc.vector.tensor_add(
                            acc3[:, :, s0:TQB],
                            acc3[:, :, s0:TQB],
                            ex3[:, :, s0:TQB])
                    # P @ V: both heads concurrent (col groups 0/64);
                    # masked left cols skipped (zero contribution)
                    for hl in range(2):
                        nc.tensor.matmul(
                            yt[hl * hd:(hl + 1) * hd, s0:TQB],
                            lhsT=t_v[kc][
                                :, (2 * hp + hl) * hd:
                                   (2 * hp + hl + 1) * hd],
                            rhs=ex3[:, hl, s0:TQB],
                            start=(kc == 0),
                            stop=(kc == n_kc - 1),
                            skip_group_check=True,
                        )
                    # drip-feed one deferred work unit per kc
                    pop_pending(tags)
                # denominators: paired 1-row ones-matmuls reduce both
                # accumulators over partitions into PSUM rows 0/64, then
                # stage (ScalarE), EA broadcast, reciprocal + scale on DVE
                dns = pz.tile([P, TQB], F32, tag="zp")
                for hl in range(2):
                    nc.tensor.matmul(
                        dns[hl * hd:hl * hd + 1, :],
                        lhsT=t_ones,
                        rhs=acc[:, hl * TQB:(hl + 1) * TQB],
                        skip_group_check=True,
                    )
                with nc.allow_low_precision(reason="bf16 softmax denom"):
                    nc.vector.tensor_copy(t_scp[0:1, :], dns[0:1, :])
                    nc.vector.tensor_copy(
                        t_scp[hd:hd + 1, :], dns[hd:hd + 1, :])
                if gi + 1 < len(groups):
                    def make_fin(yt=yt, qb=qb, hp=hp):
                        def fin():
                            bc = pz.tile([P, TQB], F32, tag="zp")
                            nc.tensor.matmul(
                                bc[:], lhsT=t_ea, rhs=t_scp[:])
                            rcpb = prcp.tile([P, TQB], F32, tag="rcpb")
                            nc.vector.reciprocal_approx_fast(rcpb[:], bc[:])
                            nc.vector.tensor_mul(
                                t_yn[hp][:, qb * TQB:(qb + 1) * TQB],
                                yt[:], rcpb[:])
                            if hp == 1 and qb < NQB - 1:
                                pending.extend(
                                    ('z', th) for th in zproj_thunks(qb))
                        return fin
                    fin_prev[0] = make_fin()
                else:
                    bc = pz.tile([P, TQB], F32, tag="zp")
                    nc.tensor.matmul(bc[:], lhsT=t_ea, rhs=t_scp[:])
                    rcpb = prcp.tile([P, TQB], F32, tag="rcpb")
                    nc.vector.reciprocal_approx_fast(rcpb[:], bc[:])
                    # final group: normalize, project and write out z per
                    # 128-row block so the tail pipelines instead of
                    # serializing
                    while pending:
                        pending.pop(0)[1]()
                    zs = pzs.tile([P, (TQB // P) * C], F32, tag="zs")
                    for m in range(TQB // P):
                        tt = qb * (TQB // P) + m
                        nc.vector.tensor_mul(
                            t_yn[hp][:, tt * P:(tt + 1) * P],
                            yt[:, m * P:(m + 1) * P],
                            rcpb[:, m * P:(m + 1) * P])
                        for co in range(C // TQB):
                            zps = pz.tile([P, TQB], F32, tag="zp")
                            for jc in range(NJC):
                                nc.tensor.matmul(
                                    zps[:],
                                    lhsT=t_yn[jc][:, tt * P:(tt + 1) * P],
                                    rhs=t_wp[:, jc * C + co * TQB:
                                             jc * C + (co + 1) * TQB],
                                    start=(jc == 0),
                                    stop=(jc == NJC - 1),
                                )
                            eng = nc.vector.tensor_copy if co == 0 \
                                else nc.scalar.copy
                            eng(
                                zs[:, (m * (C // TQB) + co) * TQB:
                                   (m * (C // TQB) + co + 1) * TQB],
                                zps[:])
                        nc.gpsimd.dma_start(
                            z[tt * P:(tt + 1) * P, :],
                            zs[:, m * C:(m + 1) * C],
                        )


_ROPE_PERM = np.concatenate([np.arange(0, 64, 2), np.arange(1, 64, 2)])


def _shuf(w):
    """[C, N] -> [128, (C//128)*N]: chunk k of 128 rows -> cols [k*N,(k+1)*N)."""
    C, N = w.shape
    return np.ascontiguousarray(
        w.reshape(C // 128, 128, N).transpose(1, 0, 2).reshape(128, -1))


def _host_inputs(x_b, Wq, bq, Wk, bk, Wv, bv, Wp, heads, T, C, hd):
    """Build the per-core DRAM input dict (numpy)."""
    HD = len(heads) * hd
    rows = np.concatenate([h * hd + _ROPE_PERM for h in heads])
    rows_nop = np.concatenate([np.arange(h * hd, (h + 1) * hd) for h in heads])

    xT = np.ascontiguousarray(x_b.T).astype(NPBF16)
    wqs = _shuf(Wq[rows].T).astype(NPBF16)
    wks = _shuf(Wk[rows].T).astype(NPBF16)
    wvs = _shuf(Wv[rows_nop].T).astype(NPBF16)
    wps = _shuf(np.ascontiguousarray(Wp[:, rows_nop].T)).astype(NPBF16)

    j = np.arange(hd // 2, dtype=np.float64)
    inv_freq = 1.0 / (10000.0 ** (2.0 * j / hd))
    t = np.arange(T, dtype=np.float64)
    ang = t[:, None] * inv_freq[None, :]          # [T, 32]
    cosv = np.cos(ang)
    sinv = np.sin(ang)
    r = np.arange(P)
    cosq = cosv[:, r % (hd // 2)].T.astype(np.float32)
    sgn = np.where((r % hd) < hd // 2, -1.0, 1.0)
    sinsq = (sinv[:, r % (hd // 2)] * sgn[None, :]).T.astype(np.float32)

    pair = np.where((r % hd) < hd // 2, r + hd // 2, r - hd // 2)
    jmat = np.zeros((P, P), np.float32)
    jmat[pair, r] = 1.0
    ea = np.zeros((P, P), np.float32)
    ea[(r // hd) * hd, r] = 1.0
    # ntri.T @ eye adds NEG strictly above the diagonal (tk > tq)
    ntri = np.where(r[None, :] > r[:, None], NEG, 0.0).astype(np.float32)
    eye = np.eye(P, dtype=np.float32)
    ones2 = np.ones((P, 2), np.float32)

    NJC = HD // P
    bqTh = bq[rows].reshape(NJC, P).T.astype(np.float32)
    bkTh = bk[rows].reshape(NJC, P).T.astype(np.float32)
    bvb = np.tile(bv[rows_nop][None, :], (P, 1)).astype(np.float32)

    fcst = np.concatenate([bqTh, bkTh, bvb], axis=1).astype(np.float32)
    bcst = np.concatenate(
        [jmat, ea, ntri, eye, ones2], axis=1).astype(NPBF16)

    return {
        "xT": xT, "wqs": wqs, "wks": wks, "wvs": wvs, "wps": wps,
        "fcst": np.ascontiguousarray(fcst),
        "bcst": np.ascontiguousarray(bcst),
        "cosd": np.ascontiguousarray(cosq[0:32]).astype(NPBF16),
        "sind": np.ascontiguousarray(sinsq[0:64]).astype(NPBF16),
    }


def make_core_inputs(x, Wq, bq, Wk, bk, Wv, bv, Wp, T=2048, C=1024, hd=64,
                     heads_per_core=4):
    in_maps = []
    for c in range(N_CORES):
        b = c // 4
        g = c % 4
        heads = list(range(g * heads_per_core, (g + 1) * heads_per_core))
        in_maps.append(_host_inputs(
            np.asarray(x[b]), Wq, bq, Wk, bk, Wv, bv, Wp, heads, T, C, hd))
    return in_maps


def kernel(x, Wq, bq, Wk, bk, Wv, bv, Wp, bp):
    x = np.asarray(x, np.float32)
    Wq = np.asarray(Wq, np.float32)
    bq = np.asarray(bq, np.float32)
    Wk = np.asarray(Wk, np.float32)
    bk = np.asarray(bk, np.float32)
    Wv = np.asarray(Wv, np.float32)
    bv = np.asarray(bv, np.float32)
    Wp = np.asarray(Wp, np.float32)
    bp = np.asarray(bp, np.float32)
    B, T, C = x.shape

    nc = bacc.Bacc("TRN2", target_bir_lowering=False, debug=False,
                   num_devices=N_CORES)
    build_attention_kernel(
        nc, T=T, C=C,
        zero_qk_bias=not (bq.any() or bk.any()))
    nc.compile()

    in_maps = make_core_inputs(x, Wq, bq, Wk, bk, Wv, bv, Wp, T=T, C=C)
    res = run_bass_kernel_spmd(nc, in_maps, list(range(N_CORES)))

    out = np.zeros((B, T, C), np.float32)
    for c in range(N_CORES):
        out[c // 4] += res.results[c]["z"]
    out += bp[None, None, :]
    return out


if __name__ == "__main__":
    import reference

    inputs = reference.setup_inputs()
    expected = np.asarray(reference.reference(**inputs))
    actual = kernel(**{k: np.asarray(v) for k, v in inputs.items()})
    err = np.abs(actual - expected).max() / np.abs(expected).max()
    print("Relative error:", err)
